# revision 31
# baseline (speedup 1.0000x reference)
"""HeadUpdator kernel for 8 Trainium2 NeuronCores.

Math: the FFT "assembly" step reduces exactly to
    assemble[b, n, c] = sum_spatial(pred_final[b, n]) * sum_spatial(feat_final[b, c])
because irfft2(rfft2(p) * rfft2(f)) is a circular convolution, and summing a
circular convolution over all output positions factors into the product of the
operand sums.

The spatial sum of each zero-padded depthwise conv output factors as
    sum(conv(x, W)) = sum_k W_k * rect_k(x) + H*W*bias
where rect_k is the sum of x over a rectangle missing up to 5 border rows or
cols.  So the device-side work over the 256 MB `feat` tensor is a pure
streaming per-image total-sum; border corrections are computed on host from
thin slices of feat (10 rows + 10 cols + 4 corners per conv channel).

Device (per core, data-parallel over batch: 2 samples/core = 128 images):
  - feat streams as dither-quantized fp8 (see TILE_FREE comment) and is
    reduced by three engines in parallel: PE (block-ones matmuls into PSUM),
    ScalarE (activation accum) and VectorE (tensor_reduce), overlapped with
    a ~22us 16-engine DMA stream at ~400 GB/s.
  - pred: host-upsampled image -> fused sigmoid/accum ops.
Host: exact bilinear x2 upsample, fp8 error-diffusion cast, border/corner
corrections, the tiny gated MLP head (16x64 matmuls), and output assembly.
"""

import numpy as np

BS, CH, H, W = 16, 64, 256, 256
NCORES = 8
BL = BS // NCORES            # local batches per core
IMGS = BL * CH               # images per core
HW = H * W
CORE_FLOATS = IMGS * HW      # 8388608 feat elements per core
# feat streams as fp8 (e4m3) with host-side error-diffusion dithering: each
# element is rounded up/down to an adjacent fp8 value such that rounding
# errors cancel within every 64-element run, so the per-image sums the device
# computes are nearly exact (end-to-end error ~5e-3 of the output vs the
# 2e-2 tolerance; plain round-to-nearest fp8 would be 3e-2).  This quarters
# HBM traffic; the reduce itself is then the critical path, split across
# THREE engines:
#   - PE (tensor engine, 307 G elem/s): batch b=1's 64 images, host-permuted
#     into 4 "groups" of 16 images so a block-ones stationary [128,16]
#     contracts 8 chunk-partitions per image into a [16,512] PSUM tile over
#     16 accumulating matmuls; VectorE drains each PSUM (0.7us per 16 imgs).
#   - ScalarE (activation accum, 153.6 G elem/s) and VectorE (tensor_reduce,
#     122.9 G elem/s): batch b=0's 64 images as flat [128,f] tiles, split in
#     proportion to their rates, with tapered tail tiles.
VA_FLOATS = CORE_FLOATS // 2          # images 0..63 of the core (batch 0)
VA_UNITS = VA_FLOATS // 128           # 32768 free units per partition
TILE_FREE = [8192, 8192, 8192, 2048, 2048, 2048, 1024, 1024]
TILE_ENG = ['a', 'v', 'a', 'a', 'v', 'a', 'v', 'v']
TILE_OFS = np.cumsum([0] + TILE_FREE[:-1]).tolist()
TILES = len(TILE_FREE)
assert sum(TILE_FREE) == VA_UNITS
ACT_TILES = [t for t in range(TILES) if TILE_ENG[t] == 'a']  # 20480 units
VEC_TILES = [t for t in range(TILES) if TILE_ENG[t] == 'v']  # 12288 units
assert sum(TILE_FREE[t] for t in ACT_TILES) == 20480
assert sum(TILE_FREE[t] for t in VEC_TILES) == 12288
PE_GROUPS = 4                         # 4 x 16 images, batch b=1
PE_STEPS = 16                         # matmuls per group, rhs [128, 512]
LN_EPS = 1e-5

_NC_CACHE = {}
TRACE = False          # test harness sets True to collect an NTFF profile
LAST_RESULTS = None    # BassKernelResults of the most recent run


def _build_nc():
    import concourse.tile as tile
    from concourse import bacc, mybir

    f32 = mybir.dt.float32
    Act = mybir.ActivationFunctionType
    AX = mybir.AxisListType.X

    f8 = mybir.dt.float8e4
    from concourse.bass import MemorySpace

    nc = bacc.Bacc("TRN2", target_bir_lowering=False, debug=False,
                   num_devices=NCORES)
    feat = nc.dram_tensor("feat", [VA_FLOATS], f8,
                          kind="ExternalInput").ap()
    fpe = nc.dram_tensor("fpe", [PE_GROUPS, 128, PE_STEPS * 512], f8,
                         kind="ExternalInput").ap()
    ones = nc.dram_tensor("ones", [128, 16], f8, kind="ExternalInput").ap()
    f16 = mybir.dt.float16
    up = nc.dram_tensor("up", [128, BL * 512], f16,
                        kind="ExternalInput").ap()
    # outv columns: VEC_TILES totals, then PE-group image sums (rows 0:16),
    # then -sum(p1*s2) per batch; outa columns: ACT_TILES totals then
    # sum(p1) per batch
    NV = len(VEC_TILES) + PE_GROUPS + BL
    NA = len(ACT_TILES) + BL
    outv = nc.dram_tensor("outv", [128, NV], f32, kind="ExternalOutput").ap()
    outa = nc.dram_tensor("outa", [128, NA], f32, kind="ExternalOutput").ap()

    with tile.TileContext(nc) as tc:
        with (
            tc.tile_pool(name="big", bufs=3) as big,
            tc.tile_pool(name="t2k", bufs=3) as t2k,
            tc.tile_pool(name="t1k", bufs=2) as t1k,
            tc.tile_pool(name="pe", bufs=4) as pep,
            tc.tile_pool(name="psum", bufs=4, space=MemorySpace.PSUM) as psp,
            tc.tile_pool(name="small", bufs=5) as small,
            tc.tile_pool(name="acc", bufs=1) as accp,
        ):
            obufv = accp.tile([128, NV], f32, tag="obufv")
            obufa = accp.tile([128, NA], f32, tag="obufa")
            dummya = accp.tile([128, 1], f32, tag="dummya")
            on = accp.tile([128, 16], f8, tag="on")

            # pred side: image b as (128, 512) in columns b*512:(b+1)*512 of
            # one [128, 1024] block, DMA'd FIRST on the sync queue so it's on
            # the wire before the feat stream hogs the DMA engines.
            # S1 = sum(p1) accumulates directly off the first sigmoid;
            # sum(pred_add) = 2*S1 - sum(p1*s2) with the product summed by one
            # fused scalar_tensor_tensor on VectorE (host adds the pieces).
            u = accp.tile([128, BL * 512], f16, tag="u")
            nc.sync.dma_start(out=on[:], in_=ones[:])
            nc.sync.dma_start(out=u[:], in_=up[:])
            for b in range(BL):
                us = u[:, 512 * b:512 * (b + 1)]
                p1 = small.tile([128, 512], f32)
                ca = len(ACT_TILES) + b
                nc.scalar.activation(p1[:], us, Act.Sigmoid,
                                     accum_out=obufa[:, ca:ca + 1])
                s2 = small.tile([128, 512], f32)
                nc.scalar.activation(s2[:], p1[:], Act.Sigmoid)
                ps = small.tile([128, 512], f32)  # -p1*s2 (scratch)
                cv = len(VEC_TILES) + PE_GROUPS + b
                nc.vector.scalar_tensor_tensor(
                    out=ps[:], in0=s2[:], scalar=-1.0, in1=p1[:],
                    op0=mybir.AluOpType.mult, op1=mybir.AluOpType.mult,
                    accum_out=obufv[:, cv:cv + 1])

            # PE p-state warm-up: ~64 tiny matmuls on the ones tile keep the
            # tensor engine continuously busy through the startup window so
            # the real groups run at full clock (3us of activity -> 2.4 GHz)
            pw = psp.tile([16, 16], f32, tag="warm")
            for _ in range(64):
                nc.tensor.matmul(pw[:], on[:], on[:, 0:16],
                                 start=True, stop=True)

            # feat: VA tiles reduce on VectorE (tensor_reduce) / ScalarE
            # (activation Copy accum into a zero-stride dummy AP); PE groups
            # run 16 accumulating matmuls into a [16,512] PSUM tile that
            # VectorE drains.  DMA issue order interleaves the two streams.
            vcol = {t: i for i, t in enumerate(VEC_TILES)}
            acol = {t: i for i, t in enumerate(ACT_TILES)}
            pools = {8192: big, 2048: t2k, 1024: t1k}

            def va_tile(t):
                f = TILE_FREE[t]
                src = feat[128 * TILE_OFS[t]:128 * (TILE_OFS[t] + f)]
                x = pools[f].tile([128, f], f8, tag="x", name=f"x{t}")
                nc.sync.dma_start(out=x[:],
                                  in_=src.rearrange("(p f) -> p f", p=128))
                if t in acol:
                    nc.scalar.activation(
                        dummya.broadcast_to((128, f)), x[:], Act.Copy,
                        accum_out=obufa[:, acol[t]:acol[t] + 1])
                else:
                    c = vcol[t]
                    nc.vector.tensor_reduce(
                        out=obufv[:, c:c + 1], in_=x[:], axis=AX,
                        op=mybir.AluOpType.add)

            def pe_group(g):
                xg = pep.tile([128, PE_STEPS * 512], f8, tag="xg",
                              name=f"xg{g}")
                nc.sync.dma_start(out=xg[:], in_=fpe[g])
                pt = psp.tile([16, 512], f32, tag="pt", name=f"pt{g}")
                for s in range(PE_STEPS):
                    nc.tensor.matmul(pt[:], on[:], xg[:, 512 * s:512 * (s + 1)],
                                     start=(s == 0), stop=(s == PE_STEPS - 1))
                c = len(VEC_TILES) + g
                if g == PE_GROUPS - 1:
                    return pt  # caller drains on ScalarE after its last tile
                nc.vector.tensor_reduce(out=obufv[0:16, c:c + 1],
                                        in_=pt[:], axis=AX,
                                        op=mybir.AluOpType.add)

            # issue order: engines' first work arrives early, tails are small
            va_tile(0)            # A 8192
            pe_group(0)
            va_tile(1)            # V 8192
            pe_group(1)
            va_tile(2)            # A 8192
            pe_group(2)
            va_tile(3)            # A 2048
            va_tile(4)            # V 2048
            pt3 = pe_group(3)
            va_tile(5)            # A 2048
            va_tile(6)            # V 1024
            va_tile(7)            # V 1024
            # last PSUM drain on ScalarE so VectorE's small tails overlap
            c3 = len(VEC_TILES) + PE_GROUPS - 1
            nc.scalar.activation(
                dummya[0:16].broadcast_to((16, 512)), pt3[:], Act.Copy,
                accum_out=obufv[0:16, c3:c3 + 1])

            # two output DMAs on distinct queues so the tails overlap
            nc.scalar.dma_start(out=outa[:], in_=obufa[:])
            nc.sync.dma_start(out=outv[:], in_=obufv[:])

    nc.compile()
    return nc


def _dither_fp8(x):
    """Error-diffusion quantize float32 -> fp8 e4m3 along the last axis.

    q[j] = round_to_nearest_fp8(x[j] + carry); carry += x[j] - q[j].
    The carry resets every 64 elements (the caller reshapes so runs never
    cross an image row), keeping each run's summed rounding error within
    half an fp8 quantum.
    """
    import ml_dtypes
    f8 = ml_dtypes.float8_e4m3
    xf = np.ascontiguousarray(x, dtype=np.float32).reshape(-1, 64)
    q = np.empty(xf.shape, dtype=f8)
    c = np.zeros(xf.shape[0], dtype=np.float32)
    for j in range(64):
        t = xf[:, j] + c
        qj = t.astype(f8)
        q[:, j] = qj
        c = t - qj.astype(np.float32)
    return q.reshape(x.shape)


def _dither_fp8_mt(x, workers=16):
    """_dither_fp8 over the leading axis in parallel (numpy casts drop the
    GIL, so threads give a real speedup on the 256 MB feat tensor)."""
    from concurrent.futures import ThreadPoolExecutor
    import ml_dtypes
    n = x.shape[0]
    out = np.empty(x.shape, dtype=ml_dtypes.float8_e4m3)
    chunks = [(i, min(i + (n + workers - 1) // workers, n))
              for i in range(0, n, (n + workers - 1) // workers)]
    with ThreadPoolExecutor(workers) as ex:
        list(ex.map(lambda ab: out.__setitem__(
            slice(ab[0], ab[1]), _dither_fp8(x[ab[0]:ab[1]])), chunks))
    return out


def _upsample2(x):
    """Exact bilinear x2, half-pixel centers (align_corners=False), separable.

    x: (..., n) -> (..., 2n) along the last axis.
    out[2i] = 0.25*x[i-1] + 0.75*x[i]; out[2i+1] = 0.75*x[i] + 0.25*x[i+1]
    with edge clamping.
    """
    left = np.concatenate([x[..., :1], x[..., :-1]], axis=-1)
    right = np.concatenate([x[..., 1:], x[..., -1:]], axis=-1)
    even = 0.25 * left + 0.75 * x
    odd = 0.75 * x + 0.25 * right
    out = np.stack([even, odd], axis=-1)
    return out.reshape(*x.shape[:-1], 2 * x.shape[-1])


def _sigmoid(x):
    return 1.0 / (1.0 + np.exp(-x))


def _pred_add(u):
    """pred_add = p1 * (1 - sigmoid(p1)) + p1 for p1 = sigmoid(u)."""
    p1 = _sigmoid(u)
    return p1 * (2.0 - _sigmoid(p1))


def _ln(x, g, b):
    m = x.mean(-1, keepdims=True)
    v = ((x - m) ** 2).mean(-1, keepdims=True)
    return (x - m) / np.sqrt(v + LN_EPS) * g + b


def _conv3x3_sum(W3, bias, S, r_first, r_last, c_first, c_last, x00, x0w,
                 xh0, xhw):
    """Spatial sum of 3x3 zero-pad-1 cross-correlation over a 256x256 image,
    given total S, first/last row sums, first/last col sums, and corners."""
    re = [r_last, 0.0, r_first]   # excluded row sum for tap i = 0,1,2
    ce = [c_last, 0.0, c_first]
    corner = {(0, 0): xhw, (0, 2): xh0, (2, 0): x0w, (2, 2): x00}
    tot = 0.0
    for i in range(3):
        for j in range(3):
            g = S - re[i] - ce[j] + corner.get((i, j), 0.0)
            tot += W3[i, j] * g
    return tot + HW * bias


def _conv1d_sum(W11, bias, S, first5, last5):
    """Spatial sum of a 1x11 (or 11x1) zero-pad-5 cross-correlation given the
    total S and the per-line sums of the first/last 5 lines."""
    tot = 0.0
    for j in range(11):
        if j < 5:
            e = last5[j:].sum()
        elif j > 5:
            e = first5[:j - 5].sum()
        else:
            e = 0.0
        tot += W11[j] * (S - e)
    return tot + HW * bias


def kernel(**inputs):
    from concourse.bass_utils import run_bass_kernel_spmd

    feat = np.ascontiguousarray(np.asarray(inputs["feat"], dtype=np.float32))
    head = np.asarray(inputs["head"], dtype=np.float32)
    pred = np.asarray(inputs["pred"], dtype=np.float32)

    # host: exact bilinear x2 upsample of pred (16,1,128,128) -> (16,256,256)
    up = pred.reshape(BS, 128, 128)
    up = _upsample2(np.swapaxes(_upsample2(np.swapaxes(up, 1, 2)), 1, 2))
    up = np.ascontiguousarray(up, dtype=np.float32)

    if "nc" not in _NC_CACHE:
        _NC_CACHE["nc"] = _build_nc()
    nc = _NC_CACHE["nc"]

    feat8 = _dither_fp8_mt(feat.reshape(BS * CH, HW))
    onesb = np.zeros((128, 16), dtype=feat8.dtype)
    onesb[np.arange(128), np.arange(128) // 8] = 1.0
    in_maps = []
    for k in range(NCORES):
        upc = up[BL * k:BL * (k + 1)].reshape(BL, 128, 512)
        base = IMGS * k
        pe = np.empty((PE_GROUPS, 128, PE_STEPS * 512), dtype=feat8.dtype)
        for g in range(PE_GROUPS):
            imgs = feat8[base + 64 + 16 * g:base + 64 + 16 * (g + 1)]
            pe[g] = imgs.reshape(16, 16, 8, 512).transpose(0, 2, 1, 3
                                                           ).reshape(128, -1)
        in_maps.append({
            "feat": feat8[base:base + 64].reshape(VA_FLOATS),
            "fpe": pe,
            "ones": onesb,
            "up": np.ascontiguousarray(upc.transpose(1, 0, 2),
                                       dtype=np.float16
                                       ).reshape(128, BL * 512),
        })
    res = run_bass_kernel_spmd(nc, in_maps, list(range(NCORES)), trace=TRACE)
    global LAST_RESULTS
    LAST_RESULTS = res

    # decode: VA tile columns are per-partition sums of contiguous slices of
    # batch-0 images; PE drain columns hold batch-1 image sums directly in
    # rows 0:16; S_pa = 2*S1 - sum(p1*s2) from the two accum columns
    img_of = np.empty((TILES, 128), dtype=np.int64)
    for t in range(TILES):
        ps = np.arange(128)
        img_of[t] = (128 * TILE_OFS[t] + ps * TILE_FREE[t]) // HW
    S_all = np.empty((BS, CH), dtype=np.float64)   # per-image totals
    S1 = np.empty((BS,), dtype=np.float64)         # sum of p1 per batch
    S_pa = np.empty((BS,), dtype=np.float64)       # sum of pred_add per batch
    nv, na = len(VEC_TILES), len(ACT_TILES)
    for k in range(NCORES):
        ov = res.results[k]["outv"].astype(np.float64)
        oa = res.results[k]["outa"].astype(np.float64)
        cols = np.empty((TILES, 128), dtype=np.float64)
        cols[VEC_TILES] = ov[:, :nv].T
        cols[ACT_TILES] = oa[:, :na].T
        s_img = np.zeros(64, dtype=np.float64)
        np.add.at(s_img, img_of.ravel(), cols.ravel())
        S_all[BL * k] = s_img                       # batch 0: VA tiles
        for g in range(PE_GROUPS):                  # batch 1: PE groups
            S_all[BL * k + 1, 16 * g:16 * (g + 1)] = ov[0:16, nv + g]
        for b in range(BL):
            s1 = oa[:, na + b].sum()
            S1[BL * k + b] = s1
            S_pa[BL * k + b] = 2.0 * s1 + ov[:, nv + PE_GROUPS + b].sum()

    f64 = np.float64
    dw_w = np.asarray(inputs["dw_w"], f64)[0, 0]        # (3,3)
    dw_b = float(np.asarray(inputs["dw_b"], f64)[0])
    inc_hw_w = np.asarray(inputs["inc_hw_w"], f64)      # (8,1,3,3)
    inc_hw_b = np.asarray(inputs["inc_hw_b"], f64)
    inc_w_w = np.asarray(inputs["inc_w_w"], f64)        # (8,1,1,11)
    inc_w_b = np.asarray(inputs["inc_w_b"], f64)
    inc_h_w = np.asarray(inputs["inc_h_w"], f64)        # (8,1,11,1)
    inc_h_b = np.asarray(inputs["inc_h_b"], f64)

    fd = feat.astype(f64)
    # border sums for the conv channels (thin slices of feat)
    hw_r0 = fd[:, 40:48, 0, :].sum(-1)        # (16,8) first row sums
    hw_rh = fd[:, 40:48, 255, :].sum(-1)
    hw_c0 = fd[:, 40:48, :, 0].sum(-1)
    hw_ch = fd[:, 40:48, :, 255].sum(-1)
    w_c5 = fd[:, 48:56, :, 0:5].sum(2)        # (16,8,5) first-5 col sums
    w_ce = fd[:, 48:56, :, 251:256].sum(2)
    h_r5 = fd[:, 56:64, 0:5, :].sum(3)        # (16,8,5) first-5 row sums
    h_re = fd[:, 56:64, 251:256, :].sum(3)

    # S_feat[b, c]: spatial sums of feat after the Inception depthwise convs
    S_feat = np.array(S_all)
    for b in range(BS):
        for g in range(8):
            X = fd[b, 40 + g]
            S_feat[b, 40 + g] = _conv3x3_sum(
                inc_hw_w[g, 0], inc_hw_b[g], S_all[b, 40 + g],
                hw_r0[b, g], hw_rh[b, g], hw_c0[b, g], hw_ch[b, g],
                X[0, 0], X[0, 255], X[255, 0], X[255, 255])
            S_feat[b, 48 + g] = _conv1d_sum(
                inc_w_w[g, 0, 0], inc_w_b[g], S_all[b, 48 + g],
                w_c5[b, g], w_ce[b, g])
            S_feat[b, 56 + g] = _conv1d_sum(
                inc_h_w[g, 0, :, 0], inc_h_b[g], S_all[b, 56 + g],
                h_r5[b, g], h_re[b, g])

    # S_pred[b]: spatial sum of p1 + conv3x3(pred_add) + dw_b
    upd = up.astype(f64)
    S_pred = np.empty((BS,), dtype=f64)
    for b in range(BS):
        row0 = _pred_add(upd[b, 0, :])
        rowh = _pred_add(upd[b, 255, :])
        col0 = _pred_add(upd[b, :, 0])
        colh = _pred_add(upd[b, :, 255])
        S_pred[b] = S1[b] + _conv3x3_sum(
            dw_w, dw_b, S_pa[b],
            row0.sum(), rowh.sum(), col0.sum(), colh.sum(),
            row0[0], row0[255], rowh[0], rowh[255])

    # assemble + tiny gated MLP head (exact mirror of the reference)
    assemble = S_pred[:, None] * S_feat                 # (16, 64)
    headd = np.asarray(head, f64).reshape(BS, 1, CH)    # kk = 1

    lin = lambda x, w, b: x @ np.asarray(w, f64).T + np.asarray(b, f64)
    g = lambda n: np.asarray(inputs[n], f64)

    pred_feat = lin(assemble, inputs["pt_w"], inputs["pt_b"])     # (16,128)
    pf_in, pf_out = pred_feat[:, :CH], pred_feat[:, -CH:]
    head_feat = lin(headd, inputs["ht_w"], inputs["ht_b"])        # (16,1,128)
    hf_in, hf_out = head_feat[..., :CH], head_feat[..., -CH:]
    gate = hf_in * pf_in[:, None, :]
    head_gate = _sigmoid(_ln(lin(gate, inputs["hg_w"], inputs["hg_b"]),
                             g("hni_g"), g("hni_b")))
    pred_gate = _sigmoid(_ln(lin(gate, inputs["pg_w"], inputs["pg_b"]),
                             g("pni_g"), g("pni_b")))
    hf_out = _ln(hf_out, g("hno_g"), g("hno_b"))
    pf_out = _ln(pf_out, g("pno_g"), g("pno_b"))
    upd_h = pred_gate * pf_out[:, None, :] + head_gate * hf_out
    upd_h = lin(upd_h, inputs["fc_w"], inputs["fc_b"])
    upd_h = np.maximum(_ln(upd_h, g("fcn_g"), g("fcn_b")), 0.0)   # (16,1,64)
    out = upd_h.reshape(BS, 1, 1, CH).transpose(0, 1, 3, 2)
    return np.ascontiguousarray(out.reshape(BS, 1, CH, 1, 1), dtype=np.float32)



# revision 42
# speedup vs baseline: 1.0101x; 1.0101x over previous
"""HeadUpdator kernel for 8 Trainium2 NeuronCores.

Math: the FFT "assembly" step reduces exactly to
    assemble[b, n, c] = sum_spatial(pred_final[b, n]) * sum_spatial(feat_final[b, c])
because irfft2(rfft2(p) * rfft2(f)) is a circular convolution, and summing a
circular convolution over all output positions factors into the product of the
operand sums.

The spatial sum of each zero-padded depthwise conv output factors as
    sum(conv(x, W)) = sum_k W_k * rect_k(x) + H*W*bias
where rect_k is the sum of x over a rectangle missing up to 5 border rows or
cols.  So the device-side work over the 256 MB `feat` tensor is a pure
streaming per-image total-sum; border corrections are computed on host from
thin slices of feat (10 rows + 10 cols + 4 corners per conv channel).

Device (per core, data-parallel over batch: 2 samples/core = 128 images):
  - feat streams as dither-quantized fp8 (see TILE_FREE comment) and is
    reduced by three engines in parallel: PE (block-ones matmuls into PSUM),
    ScalarE (activation accum) and VectorE (tensor_reduce), overlapped with
    a ~22us 16-engine DMA stream at ~400 GB/s.
  - pred: host-upsampled image -> fused sigmoid/accum ops.
Host: exact bilinear x2 upsample, fp8 error-diffusion cast, border/corner
corrections, the tiny gated MLP head (16x64 matmuls), and output assembly.
"""

import numpy as np

BS, CH, H, W = 16, 64, 256, 256
NCORES = 8
BL = BS // NCORES            # local batches per core
IMGS = BL * CH               # images per core
HW = H * W
CORE_FLOATS = IMGS * HW      # 8388608 feat elements per core
# feat streams as fp8 (e4m3) with host-side error-diffusion dithering: each
# element is rounded up/down to an adjacent fp8 value such that rounding
# errors cancel within every 64-element run, so the per-image sums the device
# computes are nearly exact (end-to-end error ~5e-3 of the output vs the
# 2e-2 tolerance; plain round-to-nearest fp8 would be 3e-2).  This quarters
# HBM traffic; the reduce itself is then the critical path, split across
# THREE engines:
#   - PE (tensor engine, 307 G elem/s): batch b=1's 64 images, host-permuted
#     into 4 "groups" of 16 images so a block-ones stationary [128,16]
#     contracts 8 chunk-partitions per image into a [16,512] PSUM tile over
#     16 accumulating matmuls; VectorE drains each PSUM (0.7us per 16 imgs).
#   - ScalarE (activation accum, 153.6 G elem/s) and VectorE (tensor_reduce,
#     122.9 G elem/s): batch b=0's 64 images as flat [128,f] tiles, split in
#     proportion to their rates, with tapered tail tiles.
VA_FLOATS = CORE_FLOATS // 2          # images 0..63 of the core (batch 0)
VA_UNITS = VA_FLOATS // 128           # 32768 free units per partition
TILE_FREE = [8192, 8192, 8192, 2048, 2048, 2048, 1024, 1024]
TILE_ENG = ['a', 'v', 'a', 'a', 'v', 'a', 'v', 'v']
TILE_OFS = np.cumsum([0] + TILE_FREE[:-1]).tolist()
TILES = len(TILE_FREE)
assert sum(TILE_FREE) == VA_UNITS
ACT_TILES = [t for t in range(TILES) if TILE_ENG[t] == 'a']  # 20480 units
VEC_TILES = [t for t in range(TILES) if TILE_ENG[t] == 'v']  # 12288 units
assert sum(TILE_FREE[t] for t in ACT_TILES) == 20480
assert sum(TILE_FREE[t] for t in VEC_TILES) == 12288
# PE groups over batch b=1's 64 images: three 16-image groups early, two
# 8-image groups last so the final matmul chains after the stream ends are
# short (1.7us instead of 3.4us)
PE_SPEC = [(16, 16), (16, 16), (16, 16), (8, 8), (8, 8)]  # (images, steps)
NPE = len(PE_SPEC)
LN_EPS = 1e-5

_NC_CACHE = {}
TRACE = False          # test harness sets True to collect an NTFF profile
LAST_RESULTS = None    # BassKernelResults of the most recent run


def _build_nc():
    import concourse.tile as tile
    from concourse import bacc, mybir

    f32 = mybir.dt.float32
    Act = mybir.ActivationFunctionType
    AX = mybir.AxisListType.X

    f8 = mybir.dt.float8e4
    from concourse.bass import MemorySpace

    nc = bacc.Bacc("TRN2", target_bir_lowering=False, debug=False,
                   num_devices=NCORES)
    feat = nc.dram_tensor("feat", [VA_FLOATS], f8,
                          kind="ExternalInput").ap()
    fpe16 = nc.dram_tensor("fpe16", [3, 128, 16 * 512], f8,
                           kind="ExternalInput").ap()
    fpe8 = nc.dram_tensor("fpe8", [2, 128, 8 * 512], f8,
                          kind="ExternalInput").ap()
    # columns 0:16 = 16-image block-ones (p//8), 16:24 = 8-image (p//16)
    ones = nc.dram_tensor("ones", [128, 24], f8, kind="ExternalInput").ap()
    f16 = mybir.dt.float16
    up = nc.dram_tensor("up", [128, BL * 512], f16,
                        kind="ExternalInput").ap()
    # outv columns: VEC_TILES totals, then PE-group image sums for groups
    # 0..3 (rows 0:16 / 0:8), then -sum(p1*s2) per batch; outa columns:
    # ACT_TILES totals, sum(p1) per batch, then PE group 4's sums (rows 0:8)
    NV = len(VEC_TILES) + (NPE - 1) + BL
    NA = len(ACT_TILES) + BL + 1
    outv = nc.dram_tensor("outv", [128, NV], f32, kind="ExternalOutput").ap()
    outa = nc.dram_tensor("outa", [128, NA], f32, kind="ExternalOutput").ap()

    with tile.TileContext(nc) as tc:
        with (
            tc.tile_pool(name="big", bufs=3) as big,
            tc.tile_pool(name="t2k", bufs=3) as t2k,
            tc.tile_pool(name="t1k", bufs=2) as t1k,
            tc.tile_pool(name="pe", bufs=4) as pep,
            tc.tile_pool(name="psum", bufs=4, space=MemorySpace.PSUM) as psp,
            tc.tile_pool(name="small", bufs=5) as small,
            tc.tile_pool(name="acc", bufs=1) as accp,
        ):
            obufv = accp.tile([128, NV], f32, tag="obufv")
            obufa = accp.tile([128, NA], f32, tag="obufa")
            dummya = accp.tile([128, 1], f32, tag="dummya")
            on = accp.tile([128, 24], f8, tag="on")

            # pred side: image b as (128, 512) in columns b*512:(b+1)*512 of
            # one [128, 1024] block, DMA'd FIRST on the sync queue so it's on
            # the wire before the feat stream hogs the DMA engines.
            # S1 = sum(p1) accumulates directly off the first sigmoid;
            # sum(pred_add) = 2*S1 - sum(p1*s2) with the product summed by one
            # fused scalar_tensor_tensor on VectorE (host adds the pieces).
            u = accp.tile([128, BL * 512], f16, tag="u")
            nc.sync.dma_start(out=on[:], in_=ones[:])
            nc.sync.dma_start(out=u[:], in_=up[:])
            for b in range(BL):
                us = u[:, 512 * b:512 * (b + 1)]
                p1 = small.tile([128, 512], f32)
                ca = len(ACT_TILES) + b
                nc.scalar.activation(p1[:], us, Act.Sigmoid,
                                     accum_out=obufa[:, ca:ca + 1])
                s2 = small.tile([128, 512], f32)
                nc.scalar.activation(s2[:], p1[:], Act.Sigmoid)
                ps = small.tile([128, 512], f32)  # -p1*s2 (scratch)
                cv = len(VEC_TILES) + (NPE - 1) + b
                nc.vector.scalar_tensor_tensor(
                    out=ps[:], in0=s2[:], scalar=-1.0, in1=p1[:],
                    op0=mybir.AluOpType.mult, op1=mybir.AluOpType.mult,
                    accum_out=obufv[:, cv:cv + 1])

            # PE p-state warm-up: ~64 tiny matmuls on the ones tile keep the
            # tensor engine continuously busy through the startup window so
            # the real groups run at full clock (3us of activity -> 2.4 GHz)
            pw = psp.tile([16, 16], f32, tag="warm", bufs=1)
            for _ in range(64):
                nc.tensor.matmul(pw[:], on[:, 0:16], on[:, 0:16],
                                 start=True, stop=True)

            # feat: VA tiles reduce on VectorE (tensor_reduce) / ScalarE
            # (activation Copy accum into a zero-stride dummy AP); PE groups
            # run 16 accumulating matmuls into a [16,512] PSUM tile that
            # VectorE drains.  DMA issue order interleaves the two streams.
            vcol = {t: i for i, t in enumerate(VEC_TILES)}
            acol = {t: i for i, t in enumerate(ACT_TILES)}
            pools = {8192: big, 2048: t2k, 1024: t1k}

            def va_tile(t):
                f = TILE_FREE[t]
                src = feat[128 * TILE_OFS[t]:128 * (TILE_OFS[t] + f)]
                x = pools[f].tile([128, f], f8, tag="x", name=f"x{t}")
                nc.sync.dma_start(out=x[:],
                                  in_=src.rearrange("(p f) -> p f", p=128))
                if t in acol:
                    nc.scalar.activation(
                        dummya.broadcast_to((128, f)), x[:], Act.Copy,
                        accum_out=obufa[:, acol[t]:acol[t] + 1])
                else:
                    c = vcol[t]
                    nc.vector.tensor_reduce(
                        out=obufv[:, c:c + 1], in_=x[:], axis=AX,
                        op=mybir.AluOpType.add)

            def pe_group(g):
                m, steps = PE_SPEC[g]
                src = fpe16[g] if m == 16 else fpe8[g - 3]
                onm = on[:, 0:16] if m == 16 else on[:, 16:24]
                xg = pep.tile([128, steps * 512], f8, tag=f"xg{m}",
                              name=f"xg{g}", bufs=3 if m == 16 else 2)
                nc.sync.dma_start(out=xg[:], in_=src)
                pt = psp.tile([m, 512], f32, tag=f"pt{m}", name=f"pt{g}",
                              bufs=3 if m == 16 else 2)
                for s in range(steps):
                    nc.tensor.matmul(pt[:], onm, xg[:, 512 * s:512 * (s + 1)],
                                     start=(s == 0), stop=(s == steps - 1))
                if g == NPE - 1:
                    return pt  # drained on ScalarE at the end
                c = len(VEC_TILES) + g
                nc.vector.tensor_reduce(out=obufv[0:m, c:c + 1],
                                        in_=pt[:], axis=AX,
                                        op=mybir.AluOpType.add)

            # issue order: engines' first work arrives early, tails are small
            va_tile(0)            # A 8192
            pe_group(0)           # PE 16 imgs
            va_tile(1)            # V 8192
            pe_group(1)           # PE 16 imgs
            va_tile(2)            # A 8192
            pe_group(2)           # PE 16 imgs
            va_tile(3)            # A 2048
            va_tile(4)            # V 2048
            pe_group(3)           # PE 8 imgs, drained on VectorE
            va_tile(5)            # A 2048
            pt4 = pe_group(4)     # PE 8 imgs, drained on ScalarE below
            va_tile(6)            # V 1024
            va_tile(7)            # V 1024
            # last PSUM drain on ScalarE so VectorE's small tails overlap
            nc.scalar.activation(
                dummya[0:8].broadcast_to((8, 512)), pt4[:], Act.Copy,
                accum_out=obufa[0:8, NA - 1:NA])

            # two output DMAs on distinct queues so the tails overlap
            nc.scalar.dma_start(out=outa[:], in_=obufa[:])
            nc.sync.dma_start(out=outv[:], in_=obufv[:])

    nc.compile()
    return nc


def _dither_fp8(x):
    """Error-diffusion quantize float32 -> fp8 e4m3 along the last axis.

    q[j] = round_to_nearest_fp8(x[j] + carry); carry += x[j] - q[j].
    The carry resets every 64 elements (the caller reshapes so runs never
    cross an image row), keeping each run's summed rounding error within
    half an fp8 quantum.
    """
    import ml_dtypes
    f8 = ml_dtypes.float8_e4m3
    xf = np.ascontiguousarray(x, dtype=np.float32).reshape(-1, 64)
    q = np.empty(xf.shape, dtype=f8)
    c = np.zeros(xf.shape[0], dtype=np.float32)
    for j in range(64):
        t = xf[:, j] + c
        qj = t.astype(f8)
        q[:, j] = qj
        c = t - qj.astype(np.float32)
    return q.reshape(x.shape)


def _dither_fp8_mt(x, workers=16):
    """_dither_fp8 over the leading axis in parallel (numpy casts drop the
    GIL, so threads give a real speedup on the 256 MB feat tensor)."""
    from concurrent.futures import ThreadPoolExecutor
    import ml_dtypes
    n = x.shape[0]
    out = np.empty(x.shape, dtype=ml_dtypes.float8_e4m3)
    chunks = [(i, min(i + (n + workers - 1) // workers, n))
              for i in range(0, n, (n + workers - 1) // workers)]
    with ThreadPoolExecutor(workers) as ex:
        list(ex.map(lambda ab: out.__setitem__(
            slice(ab[0], ab[1]), _dither_fp8(x[ab[0]:ab[1]])), chunks))
    return out


def _upsample2(x):
    """Exact bilinear x2, half-pixel centers (align_corners=False), separable.

    x: (..., n) -> (..., 2n) along the last axis.
    out[2i] = 0.25*x[i-1] + 0.75*x[i]; out[2i+1] = 0.75*x[i] + 0.25*x[i+1]
    with edge clamping.
    """
    left = np.concatenate([x[..., :1], x[..., :-1]], axis=-1)
    right = np.concatenate([x[..., 1:], x[..., -1:]], axis=-1)
    even = 0.25 * left + 0.75 * x
    odd = 0.75 * x + 0.25 * right
    out = np.stack([even, odd], axis=-1)
    return out.reshape(*x.shape[:-1], 2 * x.shape[-1])


def _sigmoid(x):
    return 1.0 / (1.0 + np.exp(-x))


def _pred_add(u):
    """pred_add = p1 * (1 - sigmoid(p1)) + p1 for p1 = sigmoid(u)."""
    p1 = _sigmoid(u)
    return p1 * (2.0 - _sigmoid(p1))


def _ln(x, g, b):
    m = x.mean(-1, keepdims=True)
    v = ((x - m) ** 2).mean(-1, keepdims=True)
    return (x - m) / np.sqrt(v + LN_EPS) * g + b


def _conv3x3_sum(W3, bias, S, r_first, r_last, c_first, c_last, x00, x0w,
                 xh0, xhw):
    """Spatial sum of 3x3 zero-pad-1 cross-correlation over a 256x256 image,
    given total S, first/last row sums, first/last col sums, and corners."""
    re = [r_last, 0.0, r_first]   # excluded row sum for tap i = 0,1,2
    ce = [c_last, 0.0, c_first]
    corner = {(0, 0): xhw, (0, 2): xh0, (2, 0): x0w, (2, 2): x00}
    tot = 0.0
    for i in range(3):
        for j in range(3):
            g = S - re[i] - ce[j] + corner.get((i, j), 0.0)
            tot += W3[i, j] * g
    return tot + HW * bias


def _conv1d_sum(W11, bias, S, first5, last5):
    """Spatial sum of a 1x11 (or 11x1) zero-pad-5 cross-correlation given the
    total S and the per-line sums of the first/last 5 lines."""
    tot = 0.0
    for j in range(11):
        if j < 5:
            e = last5[j:].sum()
        elif j > 5:
            e = first5[:j - 5].sum()
        else:
            e = 0.0
        tot += W11[j] * (S - e)
    return tot + HW * bias


def kernel(**inputs):
    from concourse.bass_utils import run_bass_kernel_spmd

    feat = np.ascontiguousarray(np.asarray(inputs["feat"], dtype=np.float32))
    head = np.asarray(inputs["head"], dtype=np.float32)
    pred = np.asarray(inputs["pred"], dtype=np.float32)

    # host: exact bilinear x2 upsample of pred (16,1,128,128) -> (16,256,256)
    up = pred.reshape(BS, 128, 128)
    up = _upsample2(np.swapaxes(_upsample2(np.swapaxes(up, 1, 2)), 1, 2))
    up = np.ascontiguousarray(up, dtype=np.float32)

    if "nc" not in _NC_CACHE:
        _NC_CACHE["nc"] = _build_nc()
    nc = _NC_CACHE["nc"]

    feat8 = _dither_fp8_mt(feat.reshape(BS * CH, HW))
    onesb = np.zeros((128, 24), dtype=feat8.dtype)
    onesb[np.arange(128), np.arange(128) // 8] = 1.0
    onesb[np.arange(128), 16 + np.arange(128) // 16] = 1.0
    in_maps = []
    for k in range(NCORES):
        upc = up[BL * k:BL * (k + 1)].reshape(BL, 128, 512)
        base = IMGS * k
        pe16 = np.empty((3, 128, 16 * 512), dtype=feat8.dtype)
        for g in range(3):
            imgs = feat8[base + 64 + 16 * g:base + 64 + 16 * (g + 1)]
            pe16[g] = imgs.reshape(16, 16, 8, 512).transpose(0, 2, 1, 3
                                                             ).reshape(128, -1)
        pe8 = np.empty((2, 128, 8 * 512), dtype=feat8.dtype)
        for h in range(2):
            imgs = feat8[base + 112 + 8 * h:base + 112 + 8 * (h + 1)]
            pe8[h] = imgs.reshape(8, 8, 16, 512).transpose(0, 2, 1, 3
                                                           ).reshape(128, -1)
        in_maps.append({
            "feat": feat8[base:base + 64].reshape(VA_FLOATS),
            "fpe16": pe16,
            "fpe8": pe8,
            "ones": onesb,
            "up": np.ascontiguousarray(upc.transpose(1, 0, 2),
                                       dtype=np.float16
                                       ).reshape(128, BL * 512),
        })
    res = run_bass_kernel_spmd(nc, in_maps, list(range(NCORES)), trace=TRACE)
    global LAST_RESULTS
    LAST_RESULTS = res

    # decode: VA tile columns are per-partition sums of contiguous slices of
    # batch-0 images; PE drain columns hold batch-1 image sums directly in
    # rows 0:16; S_pa = 2*S1 - sum(p1*s2) from the two accum columns
    img_of = np.empty((TILES, 128), dtype=np.int64)
    for t in range(TILES):
        ps = np.arange(128)
        img_of[t] = (128 * TILE_OFS[t] + ps * TILE_FREE[t]) // HW
    S_all = np.empty((BS, CH), dtype=np.float64)   # per-image totals
    S1 = np.empty((BS,), dtype=np.float64)         # sum of p1 per batch
    S_pa = np.empty((BS,), dtype=np.float64)       # sum of pred_add per batch
    nv, na = len(VEC_TILES), len(ACT_TILES)
    for k in range(NCORES):
        ov = res.results[k]["outv"].astype(np.float64)
        oa = res.results[k]["outa"].astype(np.float64)
        cols = np.empty((TILES, 128), dtype=np.float64)
        cols[VEC_TILES] = ov[:, :nv].T
        cols[ACT_TILES] = oa[:, :na].T
        s_img = np.zeros(64, dtype=np.float64)
        np.add.at(s_img, img_of.ravel(), cols.ravel())
        S_all[BL * k] = s_img                       # batch 0: VA tiles
        for g in range(3):                          # batch 1: PE groups
            S_all[BL * k + 1, 16 * g:16 * (g + 1)] = ov[0:16, nv + g]
        S_all[BL * k + 1, 48:56] = ov[0:8, nv + 3]
        S_all[BL * k + 1, 56:64] = oa[0:8, na + BL]
        for b in range(BL):
            s1 = oa[:, na + b].sum()
            S1[BL * k + b] = s1
            S_pa[BL * k + b] = 2.0 * s1 + ov[:, nv + (NPE - 1) + b].sum()

    f64 = np.float64
    dw_w = np.asarray(inputs["dw_w"], f64)[0, 0]        # (3,3)
    dw_b = float(np.asarray(inputs["dw_b"], f64)[0])
    inc_hw_w = np.asarray(inputs["inc_hw_w"], f64)      # (8,1,3,3)
    inc_hw_b = np.asarray(inputs["inc_hw_b"], f64)
    inc_w_w = np.asarray(inputs["inc_w_w"], f64)        # (8,1,1,11)
    inc_w_b = np.asarray(inputs["inc_w_b"], f64)
    inc_h_w = np.asarray(inputs["inc_h_w"], f64)        # (8,1,11,1)
    inc_h_b = np.asarray(inputs["inc_h_b"], f64)

    fd = feat.astype(f64)
    # border sums for the conv channels (thin slices of feat)
    hw_r0 = fd[:, 40:48, 0, :].sum(-1)        # (16,8) first row sums
    hw_rh = fd[:, 40:48, 255, :].sum(-1)
    hw_c0 = fd[:, 40:48, :, 0].sum(-1)
    hw_ch = fd[:, 40:48, :, 255].sum(-1)
    w_c5 = fd[:, 48:56, :, 0:5].sum(2)        # (16,8,5) first-5 col sums
    w_ce = fd[:, 48:56, :, 251:256].sum(2)
    h_r5 = fd[:, 56:64, 0:5, :].sum(3)        # (16,8,5) first-5 row sums
    h_re = fd[:, 56:64, 251:256, :].sum(3)

    # S_feat[b, c]: spatial sums of feat after the Inception depthwise convs
    S_feat = np.array(S_all)
    for b in range(BS):
        for g in range(8):
            X = fd[b, 40 + g]
            S_feat[b, 40 + g] = _conv3x3_sum(
                inc_hw_w[g, 0], inc_hw_b[g], S_all[b, 40 + g],
                hw_r0[b, g], hw_rh[b, g], hw_c0[b, g], hw_ch[b, g],
                X[0, 0], X[0, 255], X[255, 0], X[255, 255])
            S_feat[b, 48 + g] = _conv1d_sum(
                inc_w_w[g, 0, 0], inc_w_b[g], S_all[b, 48 + g],
                w_c5[b, g], w_ce[b, g])
            S_feat[b, 56 + g] = _conv1d_sum(
                inc_h_w[g, 0, :, 0], inc_h_b[g], S_all[b, 56 + g],
                h_r5[b, g], h_re[b, g])

    # S_pred[b]: spatial sum of p1 + conv3x3(pred_add) + dw_b
    upd = up.astype(f64)
    S_pred = np.empty((BS,), dtype=f64)
    for b in range(BS):
        row0 = _pred_add(upd[b, 0, :])
        rowh = _pred_add(upd[b, 255, :])
        col0 = _pred_add(upd[b, :, 0])
        colh = _pred_add(upd[b, :, 255])
        S_pred[b] = S1[b] + _conv3x3_sum(
            dw_w, dw_b, S_pa[b],
            row0.sum(), rowh.sum(), col0.sum(), colh.sum(),
            row0[0], row0[255], rowh[0], rowh[255])

    # assemble + tiny gated MLP head (exact mirror of the reference)
    assemble = S_pred[:, None] * S_feat                 # (16, 64)
    headd = np.asarray(head, f64).reshape(BS, 1, CH)    # kk = 1

    lin = lambda x, w, b: x @ np.asarray(w, f64).T + np.asarray(b, f64)
    g = lambda n: np.asarray(inputs[n], f64)

    pred_feat = lin(assemble, inputs["pt_w"], inputs["pt_b"])     # (16,128)
    pf_in, pf_out = pred_feat[:, :CH], pred_feat[:, -CH:]
    head_feat = lin(headd, inputs["ht_w"], inputs["ht_b"])        # (16,1,128)
    hf_in, hf_out = head_feat[..., :CH], head_feat[..., -CH:]
    gate = hf_in * pf_in[:, None, :]
    head_gate = _sigmoid(_ln(lin(gate, inputs["hg_w"], inputs["hg_b"]),
                             g("hni_g"), g("hni_b")))
    pred_gate = _sigmoid(_ln(lin(gate, inputs["pg_w"], inputs["pg_b"]),
                             g("pni_g"), g("pni_b")))
    hf_out = _ln(hf_out, g("hno_g"), g("hno_b"))
    pf_out = _ln(pf_out, g("pno_g"), g("pno_b"))
    upd_h = pred_gate * pf_out[:, None, :] + head_gate * hf_out
    upd_h = lin(upd_h, inputs["fc_w"], inputs["fc_b"])
    upd_h = np.maximum(_ln(upd_h, g("fcn_g"), g("fcn_b")), 0.0)   # (16,1,64)
    out = upd_h.reshape(BS, 1, 1, CH).transpose(0, 1, 3, 2)
    return np.ascontiguousarray(out.reshape(BS, 1, CH, 1, 1), dtype=np.float32)



# revision 45
# speedup vs baseline: 1.0679x; 1.0572x over previous
"""HeadUpdator kernel for 8 Trainium2 NeuronCores.

Math: the FFT "assembly" step reduces exactly to
    assemble[b, n, c] = sum_spatial(pred_final[b, n]) * sum_spatial(feat_final[b, c])
because irfft2(rfft2(p) * rfft2(f)) is a circular convolution, and summing a
circular convolution over all output positions factors into the product of the
operand sums.

The spatial sum of each zero-padded depthwise conv output factors as
    sum(conv(x, W)) = sum_k W_k * rect_k(x) + H*W*bias
where rect_k is the sum of x over a rectangle missing up to 5 border rows or
cols.  So the device-side work over the 256 MB `feat` tensor is a pure
streaming per-image total-sum; border corrections are computed on host from
thin slices of feat (10 rows + 10 cols + 4 corners per conv channel).

Device (per core, data-parallel over batch: 2 samples/core = 128 images):
  - feat streams as dither-quantized fp8 (see TILE_FREE comment) and is
    reduced by three engines in parallel: PE (block-ones matmuls into PSUM),
    ScalarE (activation accum) and VectorE (tensor_reduce), overlapped with
    a ~22us 16-engine DMA stream at ~400 GB/s.
  - pred: host-upsampled image -> fused sigmoid/accum ops.
Host: exact bilinear x2 upsample, fp8 error-diffusion cast, border/corner
corrections, the tiny gated MLP head (16x64 matmuls), and output assembly.
"""

import numpy as np

BS, CH, H, W = 16, 64, 256, 256
NCORES = 8
BL = BS // NCORES            # local batches per core
IMGS = BL * CH               # images per core
HW = H * W
CORE_FLOATS = IMGS * HW      # 8388608 feat elements per core
# feat streams as fp8 (e4m3) with host-side error-diffusion dithering: each
# element is rounded up/down to an adjacent fp8 value such that rounding
# errors cancel within every 64-element run, so the per-image sums the device
# computes are nearly exact (end-to-end error ~5e-3 of the output vs the
# 2e-2 tolerance; plain round-to-nearest fp8 would be 3e-2).  This quarters
# HBM traffic; the reduce itself is then the critical path, split across
# THREE engines:
#   - PE (tensor engine, ~150-300 G elem/s depending on p-state): batch
#     b=1's 64 images, host-permuted into groups (3x16 + 2x8 images) so a
#     block-ones stationary contracts the chunk-partitions of each image
#     into a [16,512] (or [8,512]) PSUM tile over accumulating matmuls;
#     VectorE drains the first four PSUMs, ScalarE the last.
#   - ScalarE (activation accum, 153.6 G elem/s) and VectorE (tensor_reduce,
#     122.9 G elem/s): batch b=0's 64 images as flat [128,f] tiles, split in
#     proportion to their rates, with tapered tail tiles.
VA_FLOATS = CORE_FLOATS // 2          # images 0..63 of the core (batch 0)
VA_UNITS = VA_FLOATS // 128           # 32768 free units per partition
TILE_FREE = [8192, 8192, 8192, 2048, 2048, 2048, 1024, 1024]
TILE_ENG = ['a', 'v', 'a', 'a', 'v', 'a', 'v', 'v']
TILE_OFS = np.cumsum([0] + TILE_FREE[:-1]).tolist()
TILES = len(TILE_FREE)
assert sum(TILE_FREE) == VA_UNITS
ACT_TILES = [t for t in range(TILES) if TILE_ENG[t] == 'a']  # 20480 units
VEC_TILES = [t for t in range(TILES) if TILE_ENG[t] == 'v']  # 12288 units
assert sum(TILE_FREE[t] for t in ACT_TILES) == 20480
assert sum(TILE_FREE[t] for t in VEC_TILES) == 12288
# PE groups over batch b=1's 64 images: three 16-image groups early, two
# 8-image groups last so the final matmul chains after the stream ends are
# short (1.7us instead of 3.4us)
PE_SPEC = [(16, 16), (16, 16), (16, 16), (8, 8), (8, 8)]  # (images, steps)
NPE = len(PE_SPEC)
LN_EPS = 1e-5

_NC_CACHE = {}
TRACE = False          # test harness sets True to collect an NTFF profile
LAST_RESULTS = None    # BassKernelResults of the most recent run


def _build_nc():
    import concourse.tile as tile
    from concourse import bacc, mybir

    f32 = mybir.dt.float32
    Act = mybir.ActivationFunctionType
    AX = mybir.AxisListType.X

    f8 = mybir.dt.float8e4
    from concourse.bass import MemorySpace

    nc = bacc.Bacc("TRN2", target_bir_lowering=False, debug=False,
                   num_devices=NCORES)
    feat = nc.dram_tensor("feat", [VA_FLOATS], f8,
                          kind="ExternalInput").ap()
    fpe16 = nc.dram_tensor("fpe16", [3, 128, 16 * 512], f8,
                           kind="ExternalInput").ap()
    fpe8 = nc.dram_tensor("fpe8", [2, 128, 8 * 512], f8,
                          kind="ExternalInput").ap()
    # columns 0:16 = 16-image block-ones (p//8), 16:24 = 8-image (p//16)
    ones = nc.dram_tensor("ones", [128, 24], f8, kind="ExternalInput").ap()
    f16 = mybir.dt.float16
    up = nc.dram_tensor("up", [128, BL * 512], f16,
                        kind="ExternalInput").ap()
    # outv columns: VEC_TILES totals then -sum(p1*s2) per batch; outa
    # columns: ACT_TILES totals then sum(p1) per batch.  PE PSUM tiles ship
    # out whole (like the VA per-partition partials, the host adds them).
    NV = len(VEC_TILES) + (NPE - 1) + BL
    NA = len(ACT_TILES) + BL + 1
    outv = nc.dram_tensor("outv", [128, NV], f32, kind="ExternalOutput").ap()
    outa = nc.dram_tensor("outa", [128, NA], f32, kind="ExternalOutput").ap()

    with tile.TileContext(nc) as tc:
        with (
            tc.tile_pool(name="big", bufs=3) as big,
            tc.tile_pool(name="t2k", bufs=3) as t2k,
            tc.tile_pool(name="t1k", bufs=2) as t1k,
            tc.tile_pool(name="pe", bufs=4) as pep,
            tc.tile_pool(name="psum", bufs=4, space=MemorySpace.PSUM) as psp,
            tc.tile_pool(name="small", bufs=3) as small,
            tc.tile_pool(name="acc", bufs=1) as accp,
        ):
            obufv = accp.tile([128, NV], f32, tag="obufv")
            obufa = accp.tile([128, NA], f32, tag="obufa")
            dummya = accp.tile([128, 1], f32, tag="dummya")
            on = accp.tile([128, 24], f8, tag="on")

            # pred side: image b as (128, 512) in columns b*512:(b+1)*512 of
            # one [128, 1024] block, DMA'd FIRST on the sync queue so it's on
            # the wire before the feat stream hogs the DMA engines.
            # S1 = sum(p1) accumulates directly off the first sigmoid;
            # sum(pred_add) = 2*S1 - sum(p1*s2) with the product summed by one
            # fused scalar_tensor_tensor on VectorE (host adds the pieces).
            u = accp.tile([128, BL * 512], f16, tag="u")
            nc.sync.dma_start(out=on[:], in_=ones[:])
            nc.sync.dma_start(out=u[:], in_=up[:])
            for b in range(BL):
                us = u[:, 512 * b:512 * (b + 1)]
                p1 = small.tile([128, 512], f32)
                ca = len(ACT_TILES) + b
                nc.scalar.activation(p1[:], us, Act.Sigmoid,
                                     accum_out=obufa[:, ca:ca + 1])
                s2 = small.tile([128, 512], f32)
                nc.scalar.activation(s2[:], p1[:], Act.Sigmoid)
                ps = small.tile([128, 512], f32)  # -p1*s2 (scratch)
                cv = len(VEC_TILES) + (NPE - 1) + b
                nc.vector.scalar_tensor_tensor(
                    out=ps[:], in0=s2[:], scalar=-1.0, in1=p1[:],
                    op0=mybir.AluOpType.mult, op1=mybir.AluOpType.mult,
                    accum_out=obufv[:, cv:cv + 1])

            # feat: VA tiles reduce on VectorE (tensor_reduce) / ScalarE
            # (activation Copy accum into a zero-stride dummy AP); PE groups
            # run 16 accumulating matmuls into a [16,512] PSUM tile that
            # VectorE drains.  DMA issue order interleaves the two streams.
            vcol = {t: i for i, t in enumerate(VEC_TILES)}
            acol = {t: i for i, t in enumerate(ACT_TILES)}
            pools = {8192: big, 2048: t2k, 1024: t1k}

            def va_tile(t):
                f = TILE_FREE[t]
                src = feat[128 * TILE_OFS[t]:128 * (TILE_OFS[t] + f)]
                x = pools[f].tile([128, f], f8, tag="x", name=f"x{t}")
                nc.sync.dma_start(out=x[:],
                                  in_=src.rearrange("(p f) -> p f", p=128))
                if t in acol:
                    nc.scalar.activation(
                        dummya.broadcast_to((128, f)), x[:], Act.Copy,
                        accum_out=obufa[:, acol[t]:acol[t] + 1])
                else:
                    c = vcol[t]
                    nc.vector.tensor_reduce(
                        out=obufv[:, c:c + 1], in_=x[:], axis=AX,
                        op=mybir.AluOpType.add)

            def pe_group(g):
                m, steps = PE_SPEC[g]
                src = fpe16[g] if m == 16 else fpe8[g - 3]
                onm = on[:, 0:16] if m == 16 else on[:, 16:24]
                xg = pep.tile([128, steps * 512], f8, tag=f"xg{m}",
                              name=f"xg{g}", bufs=3 if m == 16 else 2)
                nc.sync.dma_start(out=xg[:], in_=src)
                pt = psp.tile([m, 512], f32, tag=f"pt{m}", name=f"pt{g}",
                              bufs=3 if m == 16 else 2)
                for s in range(steps):
                    nc.tensor.matmul(pt[:], onm, xg[:, 512 * s:512 * (s + 1)],
                                     start=(s == 0), stop=(s == steps - 1))
                if g == NPE - 1:
                    return pt  # drained on ScalarE at the end
                c = len(VEC_TILES) + g
                nc.vector.tensor_reduce(out=obufv[0:m, c:c + 1],
                                        in_=pt[:], axis=AX,
                                        op=mybir.AluOpType.add)

            # issue order: engines' first work arrives early, tails are small
            va_tile(0)            # A 8192
            pe_group(0)           # PE 16 imgs
            va_tile(1)            # V 8192
            pe_group(1)           # PE 16 imgs
            va_tile(2)            # A 8192
            pe_group(2)           # PE 16 imgs
            va_tile(3)            # A 2048
            va_tile(4)            # V 2048
            pe_group(3)           # PE 8 imgs, drained on VectorE
            va_tile(5)            # A 2048
            pt4 = pe_group(4)     # PE 8 imgs, drained on ScalarE below
            va_tile(6)            # V 1024
            va_tile(7)            # V 1024
            # last PSUM drain on ScalarE so VectorE's small tails overlap
            nc.scalar.activation(
                dummya[0:8].broadcast_to((8, 512)), pt4[:], Act.Copy,
                accum_out=obufa[0:8, NA - 1:NA])

            # two output DMAs on distinct queues so the tails overlap
            nc.scalar.dma_start(out=outa[:], in_=obufa[:])
            nc.sync.dma_start(out=outv[:], in_=obufv[:])

    nc.compile()
    return nc


def _dither_fp8(x):
    """Error-diffusion quantize float32 -> fp8 e4m3 along the last axis.

    q[j] = round_to_nearest_fp8(x[j] + carry); carry += x[j] - q[j].
    The carry resets every 64 elements (the caller reshapes so runs never
    cross an image row), keeping each run's summed rounding error within
    half an fp8 quantum.
    """
    import ml_dtypes
    f8 = ml_dtypes.float8_e4m3
    xf = np.ascontiguousarray(x, dtype=np.float32).reshape(-1, 64)
    q = np.empty(xf.shape, dtype=f8)
    c = np.zeros(xf.shape[0], dtype=np.float32)
    for j in range(64):
        t = xf[:, j] + c
        qj = t.astype(f8)
        q[:, j] = qj
        c = t - qj.astype(np.float32)
    return q.reshape(x.shape)


def _dither_fp8_mt(x, workers=16):
    """_dither_fp8 over the leading axis in parallel (numpy casts drop the
    GIL, so threads give a real speedup on the 256 MB feat tensor)."""
    from concurrent.futures import ThreadPoolExecutor
    import ml_dtypes
    n = x.shape[0]
    out = np.empty(x.shape, dtype=ml_dtypes.float8_e4m3)
    chunks = [(i, min(i + (n + workers - 1) // workers, n))
              for i in range(0, n, (n + workers - 1) // workers)]
    with ThreadPoolExecutor(workers) as ex:
        list(ex.map(lambda ab: out.__setitem__(
            slice(ab[0], ab[1]), _dither_fp8(x[ab[0]:ab[1]])), chunks))
    return out


def _upsample2(x):
    """Exact bilinear x2, half-pixel centers (align_corners=False), separable.

    x: (..., n) -> (..., 2n) along the last axis.
    out[2i] = 0.25*x[i-1] + 0.75*x[i]; out[2i+1] = 0.75*x[i] + 0.25*x[i+1]
    with edge clamping.
    """
    left = np.concatenate([x[..., :1], x[..., :-1]], axis=-1)
    right = np.concatenate([x[..., 1:], x[..., -1:]], axis=-1)
    even = 0.25 * left + 0.75 * x
    odd = 0.75 * x + 0.25 * right
    out = np.stack([even, odd], axis=-1)
    return out.reshape(*x.shape[:-1], 2 * x.shape[-1])


def _sigmoid(x):
    return 1.0 / (1.0 + np.exp(-x))


def _pred_add(u):
    """pred_add = p1 * (1 - sigmoid(p1)) + p1 for p1 = sigmoid(u)."""
    p1 = _sigmoid(u)
    return p1 * (2.0 - _sigmoid(p1))


def _ln(x, g, b):
    m = x.mean(-1, keepdims=True)
    v = ((x - m) ** 2).mean(-1, keepdims=True)
    return (x - m) / np.sqrt(v + LN_EPS) * g + b


def _conv3x3_sum(W3, bias, S, r_first, r_last, c_first, c_last, x00, x0w,
                 xh0, xhw):
    """Spatial sum of 3x3 zero-pad-1 cross-correlation over a 256x256 image,
    given total S, first/last row sums, first/last col sums, and corners."""
    re = [r_last, 0.0, r_first]   # excluded row sum for tap i = 0,1,2
    ce = [c_last, 0.0, c_first]
    corner = {(0, 0): xhw, (0, 2): xh0, (2, 0): x0w, (2, 2): x00}
    tot = 0.0
    for i in range(3):
        for j in range(3):
            g = S - re[i] - ce[j] + corner.get((i, j), 0.0)
            tot += W3[i, j] * g
    return tot + HW * bias


def _conv1d_sum(W11, bias, S, first5, last5):
    """Spatial sum of a 1x11 (or 11x1) zero-pad-5 cross-correlation given the
    total S and the per-line sums of the first/last 5 lines."""
    tot = 0.0
    for j in range(11):
        if j < 5:
            e = last5[j:].sum()
        elif j > 5:
            e = first5[:j - 5].sum()
        else:
            e = 0.0
        tot += W11[j] * (S - e)
    return tot + HW * bias


def kernel(**inputs):
    from concourse.bass_utils import run_bass_kernel_spmd

    feat = np.ascontiguousarray(np.asarray(inputs["feat"], dtype=np.float32))
    head = np.asarray(inputs["head"], dtype=np.float32)
    pred = np.asarray(inputs["pred"], dtype=np.float32)

    # host: exact bilinear x2 upsample of pred (16,1,128,128) -> (16,256,256)
    up = pred.reshape(BS, 128, 128)
    up = _upsample2(np.swapaxes(_upsample2(np.swapaxes(up, 1, 2)), 1, 2))
    up = np.ascontiguousarray(up, dtype=np.float32)

    if "nc" not in _NC_CACHE:
        _NC_CACHE["nc"] = _build_nc()
    nc = _NC_CACHE["nc"]

    feat8 = _dither_fp8_mt(feat.reshape(BS * CH, HW))
    onesb = np.zeros((128, 24), dtype=feat8.dtype)
    onesb[np.arange(128), np.arange(128) // 8] = 1.0
    onesb[np.arange(128), 16 + np.arange(128) // 16] = 1.0
    in_maps = []
    for k in range(NCORES):
        upc = up[BL * k:BL * (k + 1)].reshape(BL, 128, 512)
        base = IMGS * k
        pe16 = np.empty((3, 128, 16 * 512), dtype=feat8.dtype)
        for g in range(3):
            imgs = feat8[base + 64 + 16 * g:base + 64 + 16 * (g + 1)]
            pe16[g] = imgs.reshape(16, 16, 8, 512).transpose(0, 2, 1, 3
                                                             ).reshape(128, -1)
        pe8 = np.empty((2, 128, 8 * 512), dtype=feat8.dtype)
        for h in range(2):
            imgs = feat8[base + 112 + 8 * h:base + 112 + 8 * (h + 1)]
            pe8[h] = imgs.reshape(8, 8, 16, 512).transpose(0, 2, 1, 3
                                                           ).reshape(128, -1)
        in_maps.append({
            "feat": feat8[base:base + 64].reshape(VA_FLOATS),
            "fpe16": pe16,
            "fpe8": pe8,
            "ones": onesb,
            "up": np.ascontiguousarray(upc.transpose(1, 0, 2),
                                       dtype=np.float16
                                       ).reshape(128, BL * 512),
        })
    res = run_bass_kernel_spmd(nc, in_maps, list(range(NCORES)), trace=TRACE)
    global LAST_RESULTS
    LAST_RESULTS = res

    # decode: VA tile columns are per-partition sums of contiguous slices of
    # batch-0 images; PE drain columns hold batch-1 image sums directly in
    # rows 0:16; S_pa = 2*S1 - sum(p1*s2) from the two accum columns
    img_of = np.empty((TILES, 128), dtype=np.int64)
    for t in range(TILES):
        ps = np.arange(128)
        img_of[t] = (128 * TILE_OFS[t] + ps * TILE_FREE[t]) // HW
    S_all = np.empty((BS, CH), dtype=np.float64)   # per-image totals
    S1 = np.empty((BS,), dtype=np.float64)         # sum of p1 per batch
    S_pa = np.empty((BS,), dtype=np.float64)       # sum of pred_add per batch
    nv, na = len(VEC_TILES), len(ACT_TILES)
    for k in range(NCORES):
        ov = res.results[k]["outv"].astype(np.float64)
        oa = res.results[k]["outa"].astype(np.float64)
        cols = np.empty((TILES, 128), dtype=np.float64)
        cols[VEC_TILES] = ov[:, :nv].T
        cols[ACT_TILES] = oa[:, :na].T
        s_img = np.zeros(64, dtype=np.float64)
        np.add.at(s_img, img_of.ravel(), cols.ravel())
        S_all[BL * k] = s_img                       # batch 0: VA tiles
        for g in range(3):                          # batch 1: PE groups
            S_all[BL * k + 1, 16 * g:16 * (g + 1)] = ov[0:16, nv + g]
        S_all[BL * k + 1, 48:56] = ov[0:8, nv + 3]
        S_all[BL * k + 1, 56:64] = oa[0:8, na + BL]
        for b in range(BL):
            s1 = oa[:, na + b].sum()
            S1[BL * k + b] = s1
            S_pa[BL * k + b] = 2.0 * s1 + ov[:, nv + (NPE - 1) + b].sum()

    f64 = np.float64
    dw_w = np.asarray(inputs["dw_w"], f64)[0, 0]        # (3,3)
    dw_b = float(np.asarray(inputs["dw_b"], f64)[0])
    inc_hw_w = np.asarray(inputs["inc_hw_w"], f64)      # (8,1,3,3)
    inc_hw_b = np.asarray(inputs["inc_hw_b"], f64)
    inc_w_w = np.asarray(inputs["inc_w_w"], f64)        # (8,1,1,11)
    inc_w_b = np.asarray(inputs["inc_w_b"], f64)
    inc_h_w = np.asarray(inputs["inc_h_w"], f64)        # (8,1,11,1)
    inc_h_b = np.asarray(inputs["inc_h_b"], f64)

    fd = feat.astype(f64)
    # border sums for the conv channels (thin slices of feat)
    hw_r0 = fd[:, 40:48, 0, :].sum(-1)        # (16,8) first row sums
    hw_rh = fd[:, 40:48, 255, :].sum(-1)
    hw_c0 = fd[:, 40:48, :, 0].sum(-1)
    hw_ch = fd[:, 40:48, :, 255].sum(-1)
    w_c5 = fd[:, 48:56, :, 0:5].sum(2)        # (16,8,5) first-5 col sums
    w_ce = fd[:, 48:56, :, 251:256].sum(2)
    h_r5 = fd[:, 56:64, 0:5, :].sum(3)        # (16,8,5) first-5 row sums
    h_re = fd[:, 56:64, 251:256, :].sum(3)

    # S_feat[b, c]: spatial sums of feat after the Inception depthwise convs
    S_feat = np.array(S_all)
    for b in range(BS):
        for g in range(8):
            X = fd[b, 40 + g]
            S_feat[b, 40 + g] = _conv3x3_sum(
                inc_hw_w[g, 0], inc_hw_b[g], S_all[b, 40 + g],
                hw_r0[b, g], hw_rh[b, g], hw_c0[b, g], hw_ch[b, g],
                X[0, 0], X[0, 255], X[255, 0], X[255, 255])
            S_feat[b, 48 + g] = _conv1d_sum(
                inc_w_w[g, 0, 0], inc_w_b[g], S_all[b, 48 + g],
                w_c5[b, g], w_ce[b, g])
            S_feat[b, 56 + g] = _conv1d_sum(
                inc_h_w[g, 0, :, 0], inc_h_b[g], S_all[b, 56 + g],
                h_r5[b, g], h_re[b, g])

    # S_pred[b]: spatial sum of p1 + conv3x3(pred_add) + dw_b
    upd = up.astype(f64)
    S_pred = np.empty((BS,), dtype=f64)
    for b in range(BS):
        row0 = _pred_add(upd[b, 0, :])
        rowh = _pred_add(upd[b, 255, :])
        col0 = _pred_add(upd[b, :, 0])
        colh = _pred_add(upd[b, :, 255])
        S_pred[b] = S1[b] + _conv3x3_sum(
            dw_w, dw_b, S_pa[b],
            row0.sum(), rowh.sum(), col0.sum(), colh.sum(),
            row0[0], row0[255], rowh[0], rowh[255])

    # assemble + tiny gated MLP head (exact mirror of the reference)
    assemble = S_pred[:, None] * S_feat                 # (16, 64)
    headd = np.asarray(head, f64).reshape(BS, 1, CH)    # kk = 1

    lin = lambda x, w, b: x @ np.asarray(w, f64).T + np.asarray(b, f64)
    g = lambda n: np.asarray(inputs[n], f64)

    pred_feat = lin(assemble, inputs["pt_w"], inputs["pt_b"])     # (16,128)
    pf_in, pf_out = pred_feat[:, :CH], pred_feat[:, -CH:]
    head_feat = lin(headd, inputs["ht_w"], inputs["ht_b"])        # (16,1,128)
    hf_in, hf_out = head_feat[..., :CH], head_feat[..., -CH:]
    gate = hf_in * pf_in[:, None, :]
    head_gate = _sigmoid(_ln(lin(gate, inputs["hg_w"], inputs["hg_b"]),
                             g("hni_g"), g("hni_b")))
    pred_gate = _sigmoid(_ln(lin(gate, inputs["pg_w"], inputs["pg_b"]),
                             g("pni_g"), g("pni_b")))
    hf_out = _ln(hf_out, g("hno_g"), g("hno_b"))
    pf_out = _ln(pf_out, g("pno_g"), g("pno_b"))
    upd_h = pred_gate * pf_out[:, None, :] + head_gate * hf_out
    upd_h = lin(upd_h, inputs["fc_w"], inputs["fc_b"])
    upd_h = np.maximum(_ln(upd_h, g("fcn_g"), g("fcn_b")), 0.0)   # (16,1,64)
    out = upd_h.reshape(BS, 1, 1, CH).transpose(0, 1, 3, 2)
    return np.ascontiguousarray(out.reshape(BS, 1, CH, 1, 1), dtype=np.float32)

